# revision 47
# baseline (speedup 1.0000x reference)
"""Trainium2 Bass kernel for ContractiveInvertibleGNN feed-forward.

Math (reference, with group_mask == I_32):
  out[b,i] = f_i( sum_j W_adj[j,i] * g_j(X[b,j]) )
where g_j: R -> R^32 and f_i: R^32 -> R are slices of two shared MLPs
(64->128->128->32 with a residual middle block, LeakyReLU 0.01):
  g: H1 = lrelu(X[b,j]*U_j + C1_j); H2 = H1 + lrelu(H1@W2g + b2g)
     X_emb = H2 @ W3g (g_b3 folded into C2 via column sums of W_adj)
  f: Hf1 = lrelu(X_aggr@Wf1x + C2_i); Hf2 = Hf1 + lrelu(Hf1@Wf2 + bf2)
     out_i = Hf2 . V_i (+ bf3_i, added on host)

Sharding: pure data-parallel over batch across 8 cores (2048 rows each).

Per-core schedule: the 2048-row block is split into two 1024-row halves
that pipeline through three phases so the aggregation never leaves the
other engines idle:
  g(h0) | g(h1)+agg(h0) | f(h0)+agg(h1) | f(h1)
Per node j in g: DMA partition-broadcasts XT row j (bf16); h1 =
lrelu(x*U_j+C1_j) (Act, or a 3-op DVE route); four bf16 gw2 matmuls into
one psum bank; t2 = lrelu(psum) drains via Act lrelu or a 2-op single-
psum-read DVE route; H2 = h1+t2 on DVE (2x tensor_tensor) or deferred to
the PE as double-pumped gw3 accumulation. Xe[(c,d),(j,t)] (batch quarter
c stacked on partition groups) -> StreamTranspose -> Xt[(c,j),(t,d)] ->
kron(I4, W_adj) bf16 matmul -> StreamTranspose -> Xa[(c,d),(i,t)] f32 ->
DMA hop relabels f32r (only DMA may produce f32r-consumed data; DMA is
otherwise idle). f mirrors g with fw1 (f32r, padded per-chunk
stationaries), Act-biased hf1, fw2, tf drains, Hf2 merge/double-pump,
and a padded V_i stationary dotting Hf2 into psum rows 0..3; one out-DMA
per 4 nodes. A greedy balancer assigns each drain/merge/copy to
Act/DVE/PE to equalize modeled engine busy time (GpSimd cannot touch
PSUM or run generic elementwise ops, so it stays idle).
"""

import os
import sys

import numpy as np

for _p in ("/opt/trn_rl_repo", "/root/.axon_site/_ro/trn_rl_repo"):
    if os.path.isdir(_p) and _p not in sys.path:
        sys.path.insert(0, _p)

N = 32          # nodes
D = 32          # processed dim (== N, group_mask = I)
A = 128         # hidden width
B = 16384       # batch
NCORES = 8
BC = B // NCORES        # 2048 rows per core
CH = 512                # matmul free-dim chunk
NCH = BC // CH          # 4 chunks (partition-group stacking factor)
ALPHA = 0.01


def _build_program(zero_b2=True):
    from contextlib import ExitStack

    from concourse import bacc, bass, mybir, tile

    f32 = mybir.dt.float32
    f32r = mybir.dt.float32r
    bf16 = mybir.dt.bfloat16
    LRELU = mybir.ActivationFunctionType.Lrelu
    MULT = mybir.AluOpType.mult
    ADD = mybir.AluOpType.add
    MAX = mybir.AluOpType.max

    nc = bacc.Bacc("TRN2", target_bir_lowering=False, debug=False)

    def din(name, shape, dt):
        return nc.dram_tensor(
            name, list(shape), dt, kind="ExternalInput"
        ).ap()

    xt_d = din("XT", (N, BC), bf16)
    # packed constants: one DMA per dtype class to cut startup HWDGE
    # serialization. CF32 = [U | C1 | C2 | GB2 | FB2]; CBF =
    # [GW2 | FW2 | GW3P | VP | BD] (GW3P col-block c holds g_W3 at cols
    # 32c..; VP[:, (i+1)*D] = V_i; BD = kron(I4, W_adj)); FW1P row-block c
    # holds f_W1[:32] at rows 32c.. (f32r: must be DMA-produced).
    cf32_d = din("CF32", (A, 3 * N + 2), f32)
    cbf_d = din("CBF", (A, 2 * A + NCH * A + (N + 1) * D + A), bf16)
    fw1p_d = din("FW1P", (A, NCH * A), f32r)
    out_d = nc.dram_tensor("OUT", [N, BC], f32, kind="ExternalOutput").ap()

    HB = BC // 2          # 1024 rows per half
    CQ = HB // NCH        # 256 cols per quarter-chunk
    WT = 16               # t's per aggregation window

    with tile.TileContext(nc) as tc, ExitStack() as ctx:
        const = ctx.enter_context(tc.tile_pool(name="const", bufs=1))
        xbp = ctx.enter_context(tc.tile_pool(name="xbp", bufs=4))
        hp = ctx.enter_context(tc.tile_pool(name="hp", bufs=5))
        tp = ctx.enter_context(tc.tile_pool(name="tp", bufs=3))
        h2p = ctx.enter_context(tc.tile_pool(name="h2p", bufs=3))
        bigp = ctx.enter_context(tc.tile_pool(name="big", bufs=1))
        outp = ctx.enter_context(tc.tile_pool(name="outs", bufs=2))
        ppA = ctx.enter_context(tc.tile_pool(name="ppA", bufs=3, space="PSUM"))
        ppS = ctx.enter_context(tc.tile_pool(name="ppS", bufs=2, space="PSUM"))

        def load_const(ap_dram, shape):
            t = const.tile(list(shape), ap_dram.dtype,
                           tag=f"c_{ap_dram.tensor.name}")
            nc.sync.dma_start(t[:, :], ap_dram)
            return t

        cf32_s = load_const(cf32_d, (A, 3 * N + 2))
        u_s = cf32_s[:, 0:N]
        c1_s = cf32_s[:, N : 2 * N]
        c2_s = cf32_s[:, 2 * N : 3 * N]
        gb2_s = cf32_s[:, 3 * N : 3 * N + 1]
        fb2_s = cf32_s[:, 3 * N + 1 : 3 * N + 2]

        # per half s: Xe[(c,d),(j,t)] bf16; Xt[(c,j),(t,d)] bf16;
        # Xa[(c,d),(i,t)] f32 from the T2 transposes, then a DMA hop
        # relabels it f32r for the fw1 matmuls (DMA is the only legal
        # producer of f32r-consumed data; it is also nearly idle).
        xes = [bigp.tile([A, N * CQ], bf16, tag=f"xe{s}", name=f"xe{s}")
               for s in range(2)]
        xts = [bigp.tile([A, N * CQ], bf16, tag=f"xt{s}", name=f"xt{s}")
               for s in range(2)]
        xas = [bigp.tile([A, N * CQ], f32, tag="xa", name=f"xa{s}", bufs=1)
               for s in range(2)]
        xars = [bigp.tile([A, N * CQ], f32r, tag="xar", name=f"xar{s}",
                          bufs=1) for s in range(2)]

        # Greedy Act/DVE balancer (Pool/GpSimd cannot run generic
        # elementwise ops or touch PSUM on TRN2; residual merges can spill
        # to the PE as gw3/V double-pumping). Picks the option minimizing
        # the resulting max accumulated load.
        load = {"A": 0.0, "D": 0.0, "E": 0.0}

        def pick(options):
            best, bargs = None, None
            for deltas, args in options:
                m = max((load[k] + deltas.get(k, 0.0)) for k in load)
                t = max(load[k] + v for k, v in deltas.items())
                if best is None or (m, t) < best:
                    best, bargs = (m, t), (deltas, args)
            for k, v in bargs[0].items():
                load[k] += v
            return bargs[1]

        def mm(*args, **kw):
            load["E"] += args[2].free_size() * 0.4167
            nc.tensor.matmul(*args, **kw)

        nstage = [0]

        def drain_lrelu(out_ap, psum_ap, bias_ap, ncols):
            if bias_ap is not None:
                nc.scalar.activation(out_ap, psum_ap, LRELU,
                                     bias=bias_ap, alpha=ALPHA)
                load["A"] += ncols * 0.833 + 250
                return
            e = pick([({"A": ncols * 0.833 + 250}, "A"),
                      ({"D": 2 * ncols * 1.04 + 340}, "D")])
            if e == "A":
                nc.scalar.activation(out_ap, psum_ap, LRELU, alpha=ALPHA)
            else:
                # u = (alpha-1)*min(pa,0); out = pa + u (one psum read each)
                nstage[0] += 1
                s = tp.tile([A, ncols], f32, tag="sd",
                            name=f"sd{nstage[0]}")
                nc.vector.tensor_scalar(s[:, :], psum_ap, 0.0, ALPHA - 1.0,
                                        mybir.AluOpType.min, MULT)
                nc.vector.scalar_tensor_tensor(out_ap, psum_ap, 1.0, s[:, :],
                                               MULT, ADD)

        def copy_ps(out_ap, in_ap, ncols):
            e = pick([({"A": ncols * 0.833 + 250}, "A"),
                      ({"D": ncols * 1.04 + 170}, "D")])
            if e == "A":
                nc.scalar.copy(out_ap, in_ap)
            else:
                nc.vector.tensor_copy(out_ap, in_ap)

        # xbc DMA prefetch ring over the (node, half) schedule
        xbcs = {}
        SCHED = [(j, 0) for j in range(N)] + [(j, 1) for j in range(N)]
        pf = [0]

        def issue_xbc():
            if pf[0] >= len(SCHED):
                return
            j, s = SCHED[pf[0]]
            pf[0] += 1
            t = xbp.tile([A, HB], bf16, tag="xb", name=f"xbc{j}_{s}")
            xbcs[(j, s)] = t
            src = xt_d[j : j + 1, :].rearrange(
                "o (c st) -> o c st", c=NCH)[:, :, s * CQ : (s + 1) * CQ]
            nc.sync.dma_start(t[:, :], src.partition_broadcast(A))

        for _ in range(4):
            issue_xbc()

        cbf_s = load_const(cbf_d, (A, 2 * A + NCH * A + (N + 1) * D + A))
        gw2_s = cbf_s[:, 0:A]
        fw2_s = cbf_s[:, A : 2 * A]
        gw3p_s = cbf_s[:, 2 * A : 2 * A + NCH * A]
        vp_s = cbf_s[:, 2 * A + NCH * A : 2 * A + NCH * A + (N + 1) * D]
        bd_s = cbf_s[:, 2 * A + NCH * A + (N + 1) * D :]
        fw1p_s = load_const(fw1p_d, (A, NCH * A))

        h2s = {}
        h1s = {}

        def g_h1(j, s):
            # h1 is emitted one pipeline step early so the in-order Act/DVE
            # queue computes h1(j+1) while the PE runs gw2(j), instead of
            # head-blocking on the t2 drain's psum dependency.
            issue_xbc()
            h1 = hp.tile([A, HB], bf16, tag="h", name=f"h1_{j}_{s}")
            xbc = xbcs.pop((j, s))
            e = pick([({"A": HB * 0.833 + 250}, "A"),
                      ({"D": HB * 1.04 + 315}, "D")])
            if e == "A":
                nc.scalar.activation(h1[:, :], xbc[:, :],
                                     LRELU, bias=c1_s[:, j : j + 1],
                                     scale=u_s[:, j : j + 1], alpha=ALPHA)
            else:
                # z = xbc*U + C1 (4x); m2 = (alpha-1)*min(z,0) (4x);
                # h1 = z + m2 (2x)
                z = tp.tile([A, HB], bf16, tag="z", name=f"z_{j}_{s}")
                m2 = tp.tile([A, HB], bf16, tag="m2", name=f"m2_{j}_{s}")
                nc.vector.tensor_scalar(z[:, :], xbc[:, :],
                                        u_s[:, j : j + 1], c1_s[:, j : j + 1],
                                        MULT, ADD)
                nc.vector.tensor_scalar(m2[:, :], z[:, :], 0.0, ALPHA - 1.0,
                                        mybir.AluOpType.min, MULT)
                nc.vector.tensor_tensor(h1[:, :], z[:, :], m2[:, :], ADD)
            h1s[(j, s)] = h1

        def g_front(j, s, nxt=None):
            h1 = h1s.pop((j, s))
            t2 = tp.tile([A, HB], bf16, tag="t", name=f"t2_{j}_{s}")
            pa = ppA.tile([A, HB], f32, tag="pA", name=f"pag{j}{s}")
            for cc in range(NCH):
                mm(pa[:, cc * CQ : (cc + 1) * CQ], gw2_s[:, :],
                   h1[:, cc * CQ : (cc + 1) * CQ], start=True, stop=True)
            if nxt is not None:
                g_h1(nxt[0], nxt[1])
            drain_lrelu(t2[:, :], pa[:, :],
                        None if zero_b2 else gb2_s[:, 0:1], HB)
            h2s[(j, s)] = (h1, t2)

        def g_back(j, s):
            # merge emitted here (an iteration after the drain) so it never
            # head-blocks the DVE queue; the PE covers its latency with the
            # next node's gw2 matmuls queued ahead of this gw3.
            h1, t2 = h2s.pop((j, s))
            e = pick([({"D": HB * 0.52 + 110}, "D"),
                      ({"E": HB * 0.45 + 120}, "E")])
            if e == "E":
                srcs = [h1, t2]
            else:
                h2 = h2p.tile([A, HB], bf16, tag="h2", name=f"h2_{j}_{s}")
                nc.vector.tensor_tensor(h2[:, :], t2[:, :], h1[:, :], ADD)
                srcs = [h2]
            nsrc = len(srcs)
            pm3 = ppS.tile([A, 2 * CQ], f32, tag="pS", name=f"pm3_{j}_{s}")
            for c in range(NCH):
                for si, sv in enumerate(srcs):
                    mm(pm3[:, :CQ], gw3p_s[:, c * A : (c + 1) * A],
                       sv[:, c * CQ : (c + 1) * CQ],
                       start=(c == 0 and si == 0),
                       stop=(c == NCH - 1 and si == nsrc - 1))
            copy_ps(xes[s][:, j * CQ : (j + 1) * CQ], pm3[:, :CQ], CQ)

        def t1_slice(s, k):
            xe3 = xes[s].rearrange("p (j t) -> p j t", j=N).transpose(
                [0, 2, 1])
            xto = xts[s].rearrange("p (t d) -> p t d", d=D)
            st = CQ // 4
            nc.vector.transpose(xto[:, k * st : (k + 1) * st, :],
                                xe3[:, k * st : (k + 1) * st, :])
            load["D"] += st * D * 1.04 + 105

        def agg_window(s, w):
            pg = ppS.tile([A, 2 * CQ], f32, tag="pS", name=f"pg{s}_{w}")
            mm(pg[:, :], bd_s[:, :],
               xts[s][:, w * 2 * CQ : (w + 1) * 2 * CQ],
               start=True, stop=True)
            xa3 = xas[s].rearrange("p (i t) -> p i t", i=N).transpose(
                [0, 2, 1])
            nc.vector.transpose(
                xa3[:, w * WT : (w + 1) * WT, :],
                pg.rearrange("p (t d) -> p t d", d=D)[:, :, :])
            load["D"] += 2 * CQ * 1.04 + 170

        def hop(s, p):
            # f32 -> f32r relabel via DMA (quarter p of half s)
            q = N * CQ // 4
            nc.sync.dma_start(xars[s][:, p * q : (p + 1) * q],
                              xas[s].bitcast(f32r)[:, p * q : (p + 1) * q])

        # ---------------- f phase helpers ----------------
        hf1s, hf2s, osbs = {}, {}, {}

        def f_fw1(i, s):
            rhs = xars[s][:, i * CQ : (i + 1) * CQ]
            hf1s[(i, s)] = hf1 = hp.tile([A, HB], bf16, tag="h",
                                         name=f"hf1_{i}_{s}")
            pa = ppA.tile([A, HB], f32, tag="pA", name=f"paf{i}{s}")
            for cc in range(NCH):
                mm(pa[:, cc * CQ : (cc + 1) * CQ],
                   fw1p_s[:, cc * A : (cc + 1) * A], rhs,
                   start=True, stop=True)
            nc.scalar.activation(hf1[:, :], pa[:, :], LRELU,
                                 bias=c2_s[:, i : i + 1], alpha=ALPHA)
            load["A"] += HB * 0.833 + 250

        def f_fw2(i, s):
            hf1 = hf1s.pop((i, s))
            tf = tp.tile([A, HB], bf16, tag="t", name=f"tf_{i}_{s}")
            pb = ppA.tile([A, HB], f32, tag="pA", name=f"pb{i}{s}")
            for cc in range(NCH):
                mm(pb[:, cc * CQ : (cc + 1) * CQ], fw2_s[:, :],
                   hf1[:, cc * CQ : (cc + 1) * CQ], start=True, stop=True)
            drain_lrelu(tf[:, :], pb[:, :],
                        None if zero_b2 else fb2_s[:, 0:1], HB)
            hf2s[(i, s)] = (hf1, tf)

        def f_v(i, s):
            m, k = i // 4, i % 4
            if k == 0:
                osbs[(m, s)] = outp.tile([NCH, 4 * CQ], f32, tag="o",
                                         name=f"osb{m}_{s}")
            osb = osbs[(m, s)]
            hf1, tf = hf2s.pop((i, s))
            e = pick([({"D": HB * 0.52 + 110}, "D"),
                      ({"E": HB * 0.45 + 120}, "E")])
            if e == "E":
                srcs = [hf1, tf]
            else:
                hf2 = h2p.tile([A, HB], bf16, tag="h2", name=f"hf2_{i}_{s}")
                nc.vector.tensor_tensor(hf2[:, :], tf[:, :], hf1[:, :], ADD)
                srcs = [hf2]
            nsrc = len(srcs)
            pr = ppS.tile([A, 2 * CQ], f32, tag="pS", name=f"pr{i}_{s}")
            for c in range(NCH):
                base = (i + 1) * D - c
                for si, sv in enumerate(srcs):
                    mm(pr[:D, :CQ], vp_s[:, base : base + D],
                       sv[:, c * CQ : (c + 1) * CQ],
                       start=(c == 0 and si == 0),
                       stop=(c == NCH - 1 and si == nsrc - 1))
            copy_ps(osb[:, k * CQ : (k + 1) * CQ], pr[:NCH, :CQ], CQ)
            if k == 3:
                nc.sync.dma_start(
                    out_d[4 * m : 4 * m + 4, :].rearrange(
                        "i (c st) -> c i st", c=NCH)[:, :, s * CQ
                                                     : (s + 1) * CQ],
                    osb.rearrange("c (k t) -> c k t", k=4)[:, :, :])

        # ---------------- driver ----------------
        # g(h0)
        g_h1(0, 0)
        for j in range(N):
            g_front(j, 0, nxt=(j + 1, 0) if j + 1 < N else (0, 1))
            if j >= 1:
                g_back(j - 1, 0)
        g_back(N - 1, 0)
        # bridge1: g(h1) overlapped with agg(h0) + hop(h0)
        for j in range(N):
            g_front(j, 1, nxt=(j + 1, 1) if j + 1 < N else None)
            if j < 4:
                t1_slice(0, j)
            if 8 <= j < 16:
                agg_window(0, 2 * (j - 8))
                agg_window(0, 2 * (j - 8) + 1)
            if 17 <= j < 21:
                hop(0, j - 17)
            if j >= 1:
                g_back(j - 1, 1)
        g_back(N - 1, 1)
        # bridge2: f(h0) overlapped with agg(h1) + hop(h1)
        for i in range(N + 2):
            if i < N:
                f_fw1(i, 0)
            if i < 4:
                t1_slice(1, i)
            if 8 <= i < 16:
                agg_window(1, 2 * (i - 8))
                agg_window(1, 2 * (i - 8) + 1)
            if 17 <= i < 21:
                hop(1, i - 17)
            if 1 <= i <= N:
                f_fw2(i - 1, 0)
            if i >= 2:
                f_v(i - 2, 0)
        # f(h1)
        for i in range(N + 2):
            if i < N:
                f_fw1(i, 1)
            if 1 <= i <= N:
                f_fw2(i - 1, 1)
            if i >= 2:
                f_v(i - 2, 1)

    nc.compile()
    return nc


_NC_CACHE = {}


def _get_program(zero_b2=True):
    if zero_b2 not in _NC_CACHE:
        _NC_CACHE[zero_b2] = _build_program(zero_b2)
    return _NC_CACHE[zero_b2]


def _host_consts(W, embeddings, g_W1, g_b1, g_W2, g_b2, g_W3, g_b3,
                 f_W1, f_b1, f_W2, f_b2, f_W3, f_b3):
    import ml_dtypes

    f = np.float32
    bf = ml_dtypes.bfloat16
    W_adj = (W * (1.0 - np.eye(N, dtype=f))).astype(f)
    U = np.ascontiguousarray(g_W1[:D].T, dtype=f)                    # [A, N]
    C1 = np.ascontiguousarray((embeddings @ g_W1[D:] + g_b1).T, f)   # [A, N]
    s = W_adj.sum(axis=0)                                            # [N]
    C2 = (embeddings @ f_W1[D:] + f_b1 + np.outer(s, g_b3 @ f_W1[:D]))
    C2 = np.ascontiguousarray(C2.T, dtype=f)                         # [A, N]
    GW3P = np.zeros((A, NCH * A), f)
    FW1P = np.zeros((A, NCH * A), f)
    for c in range(NCH):
        GW3P[:, c * A + c * D : c * A + (c + 1) * D] = g_W3
        FW1P[c * D : (c + 1) * D, c * A : (c + 1) * A] = f_W1[:D]
    BD = np.kron(np.eye(NCH, dtype=f), W_adj).astype(f)
    VP = np.zeros((A, (N + 1) * D), f)
    for i in range(N):
        VP[:, (i + 1) * D] = f_W3[:, i]
    CF32 = np.concatenate(
        [U, C1, C2, g_b2.reshape(A, 1).astype(f),
         f_b2.reshape(A, 1).astype(f)], axis=1)
    CBF = np.concatenate(
        [np.asarray(g_W2, f), np.asarray(f_W2, f), GW3P, VP, BD], axis=1)
    return {
        "CF32": np.ascontiguousarray(CF32),
        "CBF": np.ascontiguousarray(CBF).astype(bf),
        "FW1P": FW1P,
    }


def _kernel_numpy(X, W, embeddings, g_W1, g_b1, g_W2, g_b2, g_W3, g_b3,
                  f_W1, f_b1, f_W2, f_b2, f_W3, f_b3, group_mask):
    # general fallback (non-identity group_mask)
    def lrelu(x):
        return np.where(x > 0, x, ALPHA * x)

    def mlp(x, W1, b1, W2, b2, W3, b3):
        h = lrelu(x @ W1 + b1)
        h = h + lrelu(h @ W2 + b2)
        return h @ W3 + b3

    n = W.shape[0]
    W_adj = W * (1.0 - np.eye(n, dtype=W.dtype))
    Xm = X[:, None, :] * group_mask
    E = np.broadcast_to(embeddings, (X.shape[0], n, embeddings.shape[1]))
    Xe = mlp(np.concatenate([Xm, E], 2), g_W1, g_b1, g_W2, g_b2, g_W3, g_b3)
    Xa = np.einsum("ji,bjd->bid", W_adj, Xe)
    Xr = mlp(np.concatenate([Xa, E], 2), f_W1, f_b1, f_W2, f_b2, f_W3, f_b3)
    return (Xr * group_mask).sum(axis=1).astype(np.float32)


def kernel(X, W, embeddings, g_W1, g_b1, g_W2, g_b2, g_W3, g_b3,
           f_W1, f_b1, f_W2, f_b2, f_W3, f_b3, group_mask, _run_kw=None):
    if not np.allclose(group_mask, np.eye(N, D, dtype=np.float32)):
        return _kernel_numpy(X, W, embeddings, g_W1, g_b1, g_W2, g_b2, g_W3,
                             g_b3, f_W1, f_b1, f_W2, f_b2, f_W3, f_b3,
                             group_mask)

    import ml_dtypes

    from concourse import bass_utils

    zero_b2 = not (np.any(g_b2) or np.any(f_b2))
    consts = _host_consts(W, embeddings, g_W1, g_b1, g_W2, g_b2, g_W3, g_b3,
                          f_W1, f_b1, f_W2, f_b2, f_W3, f_b3)
    XT = np.ascontiguousarray(
        np.asarray(X, np.float32).T.astype(ml_dtypes.bfloat16))  # [N, B]
    in_maps = []
    for k in range(NCORES):
        m = dict(consts)
        m["XT"] = np.ascontiguousarray(XT[:, k * BC : (k + 1) * BC])
        in_maps.append(m)

    nc = _get_program(zero_b2)
    res = bass_utils.run_bass_kernel_spmd(
        nc, in_maps, core_ids=list(range(NCORES)), **(_run_kw or {})
    )
    out = np.empty((B, D), np.float32)
    for k in range(NCORES):
        out[k * BC : (k + 1) * BC, :] = res.results[k]["OUT"].T
    out += f_b3.reshape(1, D).astype(np.float32)
    if _run_kw:
        kernel.last_results = res
    return out
